# revision 35
# baseline (speedup 1.0000x reference)
"""Trainium2 Bass kernel for a LeakyReLU RNN.

Model (B=128, S=512, I=256, H=1024, O=256):
    xproj = lrelu(x @ Wi.T + bi)                          # [B,S,H]
    h_t   = lrelu(concat(xproj_t, h_{t-1}) @ Wh.T + bh)   # recurrence over S
    out   = h_S @ Wo.T + bo                               # [B,O]

Strategy: data-parallel over batch (16 rows/core on 8 cores). Split
Wh = [Wh1 | Wh2]; U = xproj @ Wh1.T + bh is precomputed as big GEMMs,
the sequential part is h_t = lrelu(U_t + h_{t-1} @ Wh2.T) with the
hidden state as the (16-wide) stationary operand and Wh2.T streamed as
the moving operand.

The PE is instruction-issue-bound (~270ns per matmul regardless of
moving width up to 512), so the recurrence keeps only 18 PE
instructions per step: 2 identity matmuls injecting U_t into PSUM and
16 accumulating matmuls. The h-state transpose needed for the next
step's stationary operand is done off the PE with XBAR DMA transposes
(bf16), eliminating the 8 PE transposes + 8 vector copies per step
that dominated the previous version.
"""

from contextlib import ExitStack

import numpy as np

import concourse.bacc as bacc
import concourse.tile as tile
from concourse import mybir
from concourse.bass_utils import run_bass_kernel_spmd

B, S, I, H, O = 128, 512, 256, 1024, 256
NCORES = 8
BL = B // NCORES          # batch rows per core = 16
TOK = BL * S              # tokens per core = 8192
NBLK = TOK // 512         # 512-token blocks in phase 1 = 16
RING_STEPS = 8            # recurrence steps per U ring DMA
ALPHA = 0.01

F32 = mybir.dt.float32
F32R = mybir.dt.float32r
BF16 = mybir.dt.bfloat16
LRELU = mybir.ActivationFunctionType.Lrelu

_CACHED = None


def _build(S=S, NBLK=NBLK):
    TOK = BL * S
    nc = bacc.Bacc("TRN2", target_bir_lowering=False, debug=False,
                   num_devices=NCORES)

    xt_d = nc.dram_tensor("xt", [I, TOK], F32, kind="ExternalInput")
    wit_d = nc.dram_tensor("wit", [I, H], F32, kind="ExternalInput")
    wh1t_d = nc.dram_tensor("wh1t", [H, H], F32, kind="ExternalInput")
    wh2t_d = nc.dram_tensor("wh2t", [H, H], F32, kind="ExternalInput")
    wot_d = nc.dram_tensor("wot", [H, O], F32, kind="ExternalInput")
    bi_d = nc.dram_tensor("bi", [128, H // 128], F32, kind="ExternalInput")
    bh_d = nc.dram_tensor("bh", [1, H], F32, kind="ExternalInput")
    bo_d = nc.dram_tensor("bo", [1, O], F32, kind="ExternalInput")
    eye_d = nc.dram_tensor("eye128", [128, 128], F32, kind="ExternalInput")
    ident_d = nc.dram_tensor("ident", [16, 16], F32, kind="ExternalInput")
    ones_d = nc.dram_tensor("ones", [1, 128], F32, kind="ExternalInput")
    y_d = nc.dram_tensor("y", [BL, O], F32, kind="ExternalOutput")
    u_dram = nc.dram_tensor("udram", [TOK, H], F32R)

    with tile.TileContext(nc) as tc, ExitStack() as ctx:
        wpool = ctx.enter_context(tc.tile_pool(name="weights", bufs=1))
        xtpool = ctx.enter_context(tc.tile_pool(name="xt", bufs=3))
        apool = ctx.enter_context(tc.tile_pool(name="atiles", bufs=2))
        upool = ctx.enter_context(tc.tile_pool(name="usb", bufs=4))
        ringpool = ctx.enter_context(tc.tile_pool(name="uring", bufs=4))
        hpool = ctx.enter_context(tc.tile_pool(name="hbuf", bufs=2))
        opool = ctx.enter_context(tc.tile_pool(name="osb", bufs=1))
        ps1ctx = ExitStack()
        psA = ps1ctx.enter_context(tc.tile_pool(name="psA", bufs=2, space="PSUM"))
        psU = ps1ctx.enter_context(tc.tile_pool(name="psU", bufs=4, space="PSUM"))

        # ---- resident weights (gpsimd DMA casts f32 -> f32r/bf16) ----
        def wload(src, shape, tag, dt=F32R):
            t = wpool.tile(shape, dt, tag=tag, name=tag)
            nc.gpsimd.dma_start(t[:], src)
            return t

        wit = [wload(wit_d.ap()[128 * k:128 * (k + 1), :], [128, H], f"wit{k}")
               for k in range(2)]
        wh1t = [wload(wh1t_d.ap()[128 * k:128 * (k + 1), :], [128, H], f"wh1t{k}")
                for k in range(8)]
        wh2t = [wload(wh2t_d.ap()[128 * k:128 * (k + 1), :], [128, H],
                      f"wh2t{k}", dt=BF16)
                for k in range(8)]
        wot = [wload(wot_d.ap()[128 * k:128 * (k + 1), :], [128, O],
                     f"wot{k}", dt=BF16)
               for k in range(8)]
        eye = wload(eye_d.ap(), [128, 128], "eye")
        identb = wload(ident_d.ap(), [16, 16], "identb", dt=BF16)
        bh2 = wload(bh_d.ap(), [1, H], "bh2")
        bo2 = wload(bo_d.ap(), [1, O], "bo2")
        ones = wload(ones_d.ap(), [1, 128], "ones")
        bi = wpool.tile([128, H // 128], F32, tag="bi", name="bi")
        nc.sync.dma_start(bi[:], bi_d.ap())

        # ---- phase 1: A_T = lrelu(WiT.T @ Xt + bi); U = A @ Wh1.T + bh ----
        for blk in range(NBLK):
            c0 = 512 * blk
            xt = [xtpool.tile([128, 512], F32R, tag=f"xt{k}", name=f"xt{k}_{blk}") for k in range(2)]
            for k in range(2):
                nc.gpsimd.dma_start(
                    xt[k][:], xt_d.ap()[128 * k:128 * (k + 1), c0:c0 + 512])
            a = []
            for m in range(8):
                pa = psA.tile([128, 512], F32, tag="psA", name=f"psA_{blk}_{m}")
                nc.tensor.matmul(pa[:], wit[0][:, 128 * m:128 * (m + 1)],
                                 xt[0][:], start=True, stop=False)
                nc.tensor.matmul(pa[:], wit[1][:, 128 * m:128 * (m + 1)],
                                 xt[1][:], start=False, stop=True)
                am = apool.tile([128, 512], F32R, tag=f"a{m}", name=f"a{m}_{blk}")
                nc.scalar.activation(am[:], pa[:], LRELU,
                                     bias=bi[:, m:m + 1], scale=1.0, alpha=ALPHA)
                a.append(am)
            for q in range(4):
                pu = [psU.tile([128, 512], F32, tag="psU", name=f"psU_{blk}_{q}_{n}")
                      for n in range(2)]
                for n in range(2):
                    nc.tensor.matmul(pu[n][:], ones[0:1, 0:128],
                                     bh2[0:1, 512 * n:512 * (n + 1)],
                                     start=True, stop=False)
                for k in range(8):
                    for n in range(2):
                        nc.tensor.matmul(
                            pu[n][:], a[k][:, 128 * q:128 * (q + 1)],
                            wh1t[k][:, 512 * n:512 * (n + 1)],
                            start=False, stop=(k == 7))
                for n in range(2):
                    usb = upool.tile([128, 512], F32R, tag="usb", name=f"usb_{blk}_{q}_{n}")
                    nc.vector.tensor_copy(usb[:], pu[n][:])
                    nc.sync.dma_start(
                        u_dram.ap()[c0 + 128 * q:c0 + 128 * (q + 1),
                                    512 * n:512 * (n + 1)], usb[:])

        # ---- phase 2: recurrence ----
        ps1ctx.close()
        psR = ctx.enter_context(tc.tile_pool(name="psR", bufs=4, space="PSUM"))
        psTr = ctx.enter_context(tc.tile_pool(name="psTr", bufs=4, space="PSUM"))
        # h state transposed. Chunks 0-3 (from ps0, which stops early) ride
        # the long-latency XBAR DMA transpose as one [128, 4, 16] quad-tile;
        # chunks 4-7 (from ps1, stopping at step end) use short-latency PE
        # transposes + DVE copies into [128, 16] tiles so the next step's
        # matmuls aren't left waiting on a DMA semaphore.
        hTpE = []
        for e in range(2):
            te = hpool.tile([128, 2, 16], BF16, tag=f"hTpE{e}",
                            name=f"hTpE{e}_init")
            nc.gpsimd.memset(te[:].bitcast(F32), 0.0)
            hTpE.append(te)
        hTL = []
        for j in range(4, 8):
            tl = hpool.tile([128, 16], BF16, tag=f"hTL{j}", name=f"hTL{j}_init")
            nc.gpsimd.memset(tl[:].bitcast(F32), 0.0)
            hTL.append(tl)



        ring = None
        ps_next = None

        def emit_id_mms(t):
            g = t % RING_STEPS
            sel = eye[:, 16 * g:16 * (g + 1)]
            ps0 = psR.tile([16, 512], F32, tag="psR", name=f"psR0_{t}")
            ps1 = psR.tile([16, 512], F32, tag="psR", name=f"psR1_{t}")
            nc.tensor.matmul(ps0[:], sel, ring[:, 0:512],
                             start=True, stop=False)
            nc.tensor.matmul(ps1[:], sel, ring[:, 512:1024],
                             start=True, stop=False)
            return ps0, ps1

        def load_ring(t):
            ring_new = ringpool.tile([RING_STEPS * BL, H], F32R, tag="ring", name=f"ring_{t}")
            r0 = t * BL
            nc.sync.dma_start(ring_new[:], u_dram.ap()[r0:r0 + RING_STEPS * BL, :])
            return ring_new

        for t in range(S):
            if t == 0:
                ring = load_ring(0)
                ps0, ps1 = emit_id_mms(0)
            else:
                ps0, ps1 = ps_next
            # ps0 chain first (stops early -> feeds the XBAR path), then ps1
            for k in range(8):
                nc.tensor.matmul(ps0[:],
                                 hTpE[k // 2][:, k % 2, :] if k < 4
                                 else hTL[k - 4][:],
                                 wh2t[k][:, 0:512],
                                 start=False, stop=(k == 7))
            for k in range(8):
                nc.tensor.matmul(ps1[:],
                                 hTpE[k // 2][:, k % 2, :] if k < 4
                                 else hTL[k - 4][:],
                                 wh2t[k][:, 512:1024],
                                 start=False, stop=(k == 7))
            if t + 1 < S:
                nxt = t + 1
                if nxt % RING_STEPS == 0:
                    ring = load_ring(nxt)
                ps_next = emit_id_mms(nxt)
            # chunks 0-3: 2x (ACT [16,256] -> XBAR pair transpose), off the PE;
            # splitting halves the latency to the first chunks' readiness
            hTpE_new = []
            for e in range(2):
                hnE = hpool.tile([16, 256], BF16, tag=f"hnE{e}",
                                 name=f"hnE{e}_{t}")
                nc.scalar.activation(hnE[:], ps0[:, 256 * e:256 * (e + 1)],
                                     LRELU, bias=0.0, scale=1.0, alpha=ALPHA)
                he = hpool.tile([128, 2, 16], BF16, tag=f"hTpE{e}",
                                name=f"hTpE{e}_{t}")
                nc.scalar.dma_start_transpose(he[:], hnE[:])
                hTpE_new.append(he)
            hTpE = hTpE_new
            # chunks 4-7: 2x ACT [16,256] -> 4 PE transposes + DVE copies
            hTL_new = []
            for c in range(2):
                hnL = hpool.tile([16, 256], BF16, tag=f"hnL{c}",
                                 name=f"hnL{c}_{t}")
                nc.scalar.activation(hnL[:], ps1[:, 256 * c:256 * (c + 1)],
                                     LRELU, bias=0.0, scale=1.0, alpha=ALPHA)
                for u in range(2):
                    j = 4 + 2 * c + u
                    pt = psTr.tile([128, 16], BF16, tag="psTr",
                                   name=f"psTr{j}_{t}")
                    nc.tensor.transpose(pt[:], hnL[:, 128 * u:128 * (u + 1)],
                                        identb[0:16, 0:16])
                    tl = hpool.tile([128, 16], BF16, tag=f"hTL{j}",
                                    name=f"hTL{j}_{t}")
                    nc.vector.tensor_copy(tl[:], pt[:])
                    hTL_new.append(tl)
            hTL = hTL_new

        # ---- phase 3: out = h_S @ Wo.T + bo ----
        po = psR.tile([16, 512], F32, tag="psR", name="psO")
        nc.tensor.matmul(po[:, 0:O], ones[0:1, 0:16], bo2[0:1, :],
                         start=True, stop=False)
        for k in range(8):
            nc.tensor.matmul(po[:, 0:O],
                             hTpE[k // 2][:, k % 2, :] if k < 4
                             else hTL[k - 4][:],
                             wot[k][:],
                             start=False, stop=(k == 7))
        osb = opool.tile([16, O], F32, tag="osb", name="osb")
        nc.vector.tensor_copy(osb[:], po[:, 0:O])
        nc.sync.dma_start(y_d.ap(), osb[:])

    nc.compile()
    return nc


def _prep_inputs(x, Wi, bi, Wh, bh, Wo, bo):
    shared = {
        "wit": np.ascontiguousarray(Wi.T),
        "wh1t": np.ascontiguousarray(Wh[:, :H].T),
        "wh2t": np.ascontiguousarray(Wh[:, H:].T),
        "wot": np.ascontiguousarray(Wo.T),
        "bi": np.ascontiguousarray(bi.reshape(H // 128, 128).T),
        "bh": bh.reshape(1, H).copy(),
        "bo": bo.reshape(1, O).copy(),
        "eye128": np.eye(128, dtype=np.float32),
        "ident": np.eye(16, dtype=np.float32),
        "ones": np.ones((1, 128), np.float32),
    }
    in_maps = []
    for c in range(NCORES):
        xc = x[BL * c:BL * (c + 1)]            # [16, S, I]
        xt = np.ascontiguousarray(
            xc.transpose(2, 1, 0).reshape(I, TOK))  # [I, S*16] col = t*16+b
        m = dict(shared)
        m["xt"] = xt
        in_maps.append(m)
    return in_maps


def kernel(x, Wi, bi, Wh, bh, Wo, bo, _trace=False):
    global _CACHED
    x = np.asarray(x, dtype=np.float32)
    if _CACHED is None:
        _CACHED = _build()
    nc = _CACHED
    in_maps = _prep_inputs(np.asarray(x, np.float32), np.asarray(Wi, np.float32),
                           np.asarray(bi, np.float32), np.asarray(Wh, np.float32),
                           np.asarray(bh, np.float32), np.asarray(Wo, np.float32),
                           np.asarray(bo, np.float32))
    res = run_bass_kernel_spmd(nc, in_maps, list(range(NCORES)), trace=_trace)
    out = np.concatenate([res.results[c]["y"] for c in range(NCORES)], axis=0)
    if _trace:
        return out, res
    return out


# revision 36
# speedup vs baseline: 1.2736x; 1.2736x over previous
"""Trainium2 Bass kernel for a LeakyReLU RNN.

Model (B=128, S=512, I=256, H=1024, O=256):
    xproj = lrelu(x @ Wi.T + bi)                          # [B,S,H]
    h_t   = lrelu(concat(xproj_t, h_{t-1}) @ Wh.T + bh)   # recurrence over S
    out   = h_S @ Wo.T + bo                               # [B,O]

Strategy: data-parallel over batch (16 rows/core on 8 cores). Split
Wh = [Wh1 | Wh2]; U = xproj @ Wh1.T + bh is precomputed as big GEMMs,
the sequential part is h_t = lrelu(U_t + h_{t-1} @ Wh2.T) with the
hidden state as the (16-wide) stationary operand and Wh2.T streamed as
the moving operand.

The PE is instruction-issue-bound (~270ns per matmul regardless of
moving width up to 512), so the recurrence keeps only 18 PE
instructions per step: 2 identity matmuls injecting U_t into PSUM and
16 accumulating matmuls. The h-state transpose needed for the next
step's stationary operand is done off the PE with XBAR DMA transposes
(bf16), eliminating the 8 PE transposes + 8 vector copies per step
that dominated the previous version.
"""

from contextlib import ExitStack

import numpy as np

import concourse.bacc as bacc
import concourse.tile as tile
from concourse import mybir
from concourse.bass_utils import run_bass_kernel_spmd

B, S, I, H, O = 128, 512, 256, 1024, 256
NCORES = 8
BL = B // NCORES          # batch rows per core = 16
TOK = BL * S              # tokens per core = 8192
NBLK = TOK // 512         # 512-token blocks in phase 1 = 16
RING_STEPS = 8            # recurrence steps per U ring DMA
ALPHA = 0.01

F32 = mybir.dt.float32
F32R = mybir.dt.float32r
BF16 = mybir.dt.bfloat16
LRELU = mybir.ActivationFunctionType.Lrelu

_CACHED = None


def _build(S=S, NBLK=NBLK):
    TOK = BL * S
    nc = bacc.Bacc("TRN2", target_bir_lowering=False, debug=False,
                   num_devices=NCORES)

    xt_d = nc.dram_tensor("xt", [I, TOK], F32, kind="ExternalInput")
    wit_d = nc.dram_tensor("wit", [I, H], F32, kind="ExternalInput")
    wh1t_d = nc.dram_tensor("wh1t", [H, H], F32, kind="ExternalInput")
    wh2t_d = nc.dram_tensor("wh2t", [H, H], F32, kind="ExternalInput")
    wot_d = nc.dram_tensor("wot", [H, O], F32, kind="ExternalInput")
    bi_d = nc.dram_tensor("bi", [128, H // 128], F32, kind="ExternalInput")
    bh_d = nc.dram_tensor("bh", [1, H], F32, kind="ExternalInput")
    bo_d = nc.dram_tensor("bo", [1, O], F32, kind="ExternalInput")
    eye_d = nc.dram_tensor("eye128", [128, 128], F32, kind="ExternalInput")
    ident_d = nc.dram_tensor("ident", [16, 16], F32, kind="ExternalInput")
    ones_d = nc.dram_tensor("ones", [1, 128], F32, kind="ExternalInput")
    y_d = nc.dram_tensor("y", [BL, O], F32, kind="ExternalOutput")
    u_dram = nc.dram_tensor("udram", [TOK, H], F32R)

    with tile.TileContext(nc) as tc, ExitStack() as ctx:
        wpool = ctx.enter_context(tc.tile_pool(name="weights", bufs=1))
        xtpool = ctx.enter_context(tc.tile_pool(name="xt", bufs=3))
        apool = ctx.enter_context(tc.tile_pool(name="atiles", bufs=2))
        upool = ctx.enter_context(tc.tile_pool(name="usb", bufs=4))
        ringpool = ctx.enter_context(tc.tile_pool(name="uring", bufs=4))
        hpool = ctx.enter_context(tc.tile_pool(name="hbuf", bufs=2))
        opool = ctx.enter_context(tc.tile_pool(name="osb", bufs=1))
        ps1ctx = ExitStack()
        psA = ps1ctx.enter_context(tc.tile_pool(name="psA", bufs=2, space="PSUM"))
        psU = ps1ctx.enter_context(tc.tile_pool(name="psU", bufs=4, space="PSUM"))

        # ---- resident weights (gpsimd DMA casts f32 -> f32r/bf16) ----
        def wload(src, shape, tag, dt=F32R):
            t = wpool.tile(shape, dt, tag=tag, name=tag)
            nc.gpsimd.dma_start(t[:], src)
            return t

        wit = [wload(wit_d.ap()[128 * k:128 * (k + 1), :], [128, H], f"wit{k}")
               for k in range(2)]
        wh1t = [wload(wh1t_d.ap()[128 * k:128 * (k + 1), :], [128, H], f"wh1t{k}")
                for k in range(8)]
        wh2t = [wload(wh2t_d.ap()[128 * k:128 * (k + 1), :], [128, H],
                      f"wh2t{k}", dt=BF16)
                for k in range(8)]
        wot = [wload(wot_d.ap()[128 * k:128 * (k + 1), :], [128, O],
                     f"wot{k}", dt=BF16)
               for k in range(8)]
        eye = wload(eye_d.ap(), [128, 128], "eye")
        identb = wload(ident_d.ap(), [16, 16], "identb", dt=BF16)
        bh2 = wload(bh_d.ap(), [1, H], "bh2")
        bo2 = wload(bo_d.ap(), [1, O], "bo2")
        ones = wload(ones_d.ap(), [1, 128], "ones")
        bi = wpool.tile([128, H // 128], F32, tag="bi", name="bi")
        nc.sync.dma_start(bi[:], bi_d.ap())

        # ---- phase 1: A_T = lrelu(WiT.T @ Xt + bi); U = A @ Wh1.T + bh ----
        for blk in range(NBLK):
            c0 = 512 * blk
            xt = [xtpool.tile([128, 512], F32R, tag=f"xt{k}", name=f"xt{k}_{blk}") for k in range(2)]
            for k in range(2):
                nc.gpsimd.dma_start(
                    xt[k][:], xt_d.ap()[128 * k:128 * (k + 1), c0:c0 + 512])
            a = []
            for m in range(8):
                pa = psA.tile([128, 512], F32, tag="psA", name=f"psA_{blk}_{m}")
                nc.tensor.matmul(pa[:], wit[0][:, 128 * m:128 * (m + 1)],
                                 xt[0][:], start=True, stop=False)
                nc.tensor.matmul(pa[:], wit[1][:, 128 * m:128 * (m + 1)],
                                 xt[1][:], start=False, stop=True)
                am = apool.tile([128, 512], F32R, tag=f"a{m}", name=f"a{m}_{blk}")
                nc.scalar.activation(am[:], pa[:], LRELU,
                                     bias=bi[:, m:m + 1], scale=1.0, alpha=ALPHA)
                a.append(am)
            for q in range(4):
                pu = [psU.tile([128, 512], F32, tag="psU", name=f"psU_{blk}_{q}_{n}")
                      for n in range(2)]
                for n in range(2):
                    nc.tensor.matmul(pu[n][:], ones[0:1, 0:128],
                                     bh2[0:1, 512 * n:512 * (n + 1)],
                                     start=True, stop=False)
                for k in range(8):
                    for n in range(2):
                        nc.tensor.matmul(
                            pu[n][:], a[k][:, 128 * q:128 * (q + 1)],
                            wh1t[k][:, 512 * n:512 * (n + 1)],
                            start=False, stop=(k == 7))
                for n in range(2):
                    usb = upool.tile([128, 512], F32R, tag="usb", name=f"usb_{blk}_{q}_{n}")
                    nc.vector.tensor_copy(usb[:], pu[n][:])
                    nc.sync.dma_start(
                        u_dram.ap()[c0 + 128 * q:c0 + 128 * (q + 1),
                                    512 * n:512 * (n + 1)], usb[:])

        # ---- phase 2: recurrence ----
        ps1ctx.close()
        psR = ctx.enter_context(tc.tile_pool(name="psR", bufs=4, space="PSUM"))
        psTr = ctx.enter_context(tc.tile_pool(name="psTr", bufs=4, space="PSUM"))
        # h state transposed. Chunks 0-3 (from ps0, which stops early) ride
        # the long-latency XBAR DMA transpose as one [128, 4, 16] quad-tile;
        # chunks 4-7 (from ps1, stopping at step end) use short-latency PE
        # transposes + DVE copies into [128, 16] tiles so the next step's
        # matmuls aren't left waiting on a DMA semaphore.
        hTpE = hpool.tile([128, 4, 16], BF16, tag="hTpE", name="hTpE_init")
        nc.gpsimd.memset(hTpE[:].bitcast(F32), 0.0)
        hTL = []
        for j in range(4, 8):
            tl = hpool.tile([128, 16], BF16, tag=f"hTL{j}", name=f"hTL{j}_init")
            nc.gpsimd.memset(tl[:].bitcast(F32), 0.0)
            hTL.append(tl)



        ring = None
        ps_next = None

        def emit_id_mms(t):
            g = t % RING_STEPS
            sel = eye[:, 16 * g:16 * (g + 1)]
            ps0 = psR.tile([16, 512], F32, tag="psR", name=f"psR0_{t}")
            ps1 = psR.tile([16, 512], F32, tag="psR", name=f"psR1_{t}")
            nc.tensor.matmul(ps0[:], sel, ring[:, 0:512],
                             start=True, stop=False)
            nc.tensor.matmul(ps1[:], sel, ring[:, 512:1024],
                             start=True, stop=False)
            return ps0, ps1

        def load_ring(t):
            ring_new = ringpool.tile([RING_STEPS * BL, H], F32R, tag="ring", name=f"ring_{t}")
            r0 = t * BL
            nc.sync.dma_start(ring_new[:], u_dram.ap()[r0:r0 + RING_STEPS * BL, :])
            return ring_new

        for t in range(S):
            if t == 0:
                ring = load_ring(0)
                ps0, ps1 = emit_id_mms(0)
            else:
                ps0, ps1 = ps_next
            # ps0 chain first (stops early -> feeds the XBAR path), then ps1
            for k in range(8):
                nc.tensor.matmul(ps0[:], hTpE[:, k, :] if k < 4 else hTL[k - 4][:],
                                 wh2t[k][:, 0:512],
                                 start=False, stop=(k == 7))
            for k in range(8):
                nc.tensor.matmul(ps1[:], hTpE[:, k, :] if k < 4 else hTL[k - 4][:],
                                 wh2t[k][:, 512:1024],
                                 start=False, stop=(k == 7))
            if t + 1 < S:
                nxt = t + 1
                if nxt % RING_STEPS == 0:
                    ring = load_ring(nxt)
                ps_next = emit_id_mms(nxt)
            # chunks 0-3: ACT [16,512] -> XBAR quad transpose (off the PE)
            hnE = hpool.tile([16, 512], BF16, tag="hnE", name=f"hnE_{t}")
            nc.scalar.activation(hnE[:], ps0[:], LRELU,
                                 bias=0.0, scale=1.0, alpha=ALPHA)
            hTpE = hpool.tile([128, 4, 16], BF16, tag="hTpE", name=f"hTpE_{t}")
            nc.scalar.dma_start_transpose(hTpE[:], hnE[:])
            # chunks 4-7: 2x ACT [16,256] -> 4 PE transposes + DVE copies
            hTL_new = []
            for c in range(2):
                hnL = hpool.tile([16, 256], BF16, tag=f"hnL{c}",
                                 name=f"hnL{c}_{t}")
                nc.scalar.activation(hnL[:], ps1[:, 256 * c:256 * (c + 1)],
                                     LRELU, bias=0.0, scale=1.0, alpha=ALPHA)
                for u in range(2):
                    j = 4 + 2 * c + u
                    pt = psTr.tile([128, 16], BF16, tag="psTr",
                                   name=f"psTr{j}_{t}")
                    nc.tensor.transpose(pt[:], hnL[:, 128 * u:128 * (u + 1)],
                                        identb[0:16, 0:16])
                    tl = hpool.tile([128, 16], BF16, tag=f"hTL{j}",
                                    name=f"hTL{j}_{t}")
                    nc.vector.tensor_copy(tl[:], pt[:])
                    hTL_new.append(tl)
            hTL = hTL_new

        # ---- phase 3: out = h_S @ Wo.T + bo ----
        po = psR.tile([16, 512], F32, tag="psR", name="psO")
        nc.tensor.matmul(po[:, 0:O], ones[0:1, 0:16], bo2[0:1, :],
                         start=True, stop=False)
        for k in range(8):
            nc.tensor.matmul(po[:, 0:O],
                             hTpE[:, k, :] if k < 4 else hTL[k - 4][:],
                             wot[k][:],
                             start=False, stop=(k == 7))
        osb = opool.tile([16, O], F32, tag="osb", name="osb")
        nc.vector.tensor_copy(osb[:], po[:, 0:O])
        nc.sync.dma_start(y_d.ap(), osb[:])

    nc.compile()
    return nc


def _prep_inputs(x, Wi, bi, Wh, bh, Wo, bo):
    shared = {
        "wit": np.ascontiguousarray(Wi.T),
        "wh1t": np.ascontiguousarray(Wh[:, :H].T),
        "wh2t": np.ascontiguousarray(Wh[:, H:].T),
        "wot": np.ascontiguousarray(Wo.T),
        "bi": np.ascontiguousarray(bi.reshape(H // 128, 128).T),
        "bh": bh.reshape(1, H).copy(),
        "bo": bo.reshape(1, O).copy(),
        "eye128": np.eye(128, dtype=np.float32),
        "ident": np.eye(16, dtype=np.float32),
        "ones": np.ones((1, 128), np.float32),
    }
    in_maps = []
    for c in range(NCORES):
        xc = x[BL * c:BL * (c + 1)]            # [16, S, I]
        xt = np.ascontiguousarray(
            xc.transpose(2, 1, 0).reshape(I, TOK))  # [I, S*16] col = t*16+b
        m = dict(shared)
        m["xt"] = xt
        in_maps.append(m)
    return in_maps


def kernel(x, Wi, bi, Wh, bh, Wo, bo, _trace=False):
    global _CACHED
    x = np.asarray(x, dtype=np.float32)
    if _CACHED is None:
        _CACHED = _build()
    nc = _CACHED
    in_maps = _prep_inputs(np.asarray(x, np.float32), np.asarray(Wi, np.float32),
                           np.asarray(bi, np.float32), np.asarray(Wh, np.float32),
                           np.asarray(bh, np.float32), np.asarray(Wo, np.float32),
                           np.asarray(bo, np.float32))
    res = run_bass_kernel_spmd(nc, in_maps, list(range(NCORES)), trace=_trace)
    out = np.concatenate([res.results[c]["y"] for c in range(NCORES)], axis=0)
    if _trace:
        return out, res
    return out


# revision 40
# speedup vs baseline: 1.3231x; 1.0389x over previous
"""Trainium2 Bass kernel for a LeakyReLU RNN.

Model (B=128, S=512, I=256, H=1024, O=256):
    xproj = lrelu(x @ Wi.T + bi)                          # [B,S,H]
    h_t   = lrelu(concat(xproj_t, h_{t-1}) @ Wh.T + bh)   # recurrence over S
    out   = h_S @ Wo.T + bo                               # [B,O]

Strategy: data-parallel over batch (16 rows/core on 8 cores). Split
Wh = [Wh1 | Wh2]; U = xproj @ Wh1.T + bh is precomputed as big GEMMs,
the sequential part is h_t = lrelu(U_t + h_{t-1} @ Wh2.T) with the
hidden state as the (16-wide) stationary operand and Wh2.T streamed as
the moving operand.

The PE is instruction-issue-bound (~270ns per matmul regardless of
moving width up to 512), so the recurrence keeps only 18 PE
instructions per step: 2 identity matmuls injecting U_t into PSUM and
16 accumulating matmuls. The h-state transpose needed for the next
step's stationary operand is done off the PE with XBAR DMA transposes
(bf16), eliminating the 8 PE transposes + 8 vector copies per step
that dominated the previous version.
"""

from contextlib import ExitStack

import numpy as np

import concourse.bacc as bacc
import concourse.tile as tile
from concourse import mybir
from concourse.bass_utils import run_bass_kernel_spmd

B, S, I, H, O = 128, 512, 256, 1024, 256
NCORES = 8
BL = B // NCORES          # batch rows per core = 16
TOK = BL * S              # tokens per core = 8192
NBLK = TOK // 512         # 512-token blocks in phase 1 = 16
RING_STEPS = 8            # recurrence steps per U ring DMA
ALPHA = 0.01

F32 = mybir.dt.float32
F32R = mybir.dt.float32r
BF16 = mybir.dt.bfloat16
LRELU = mybir.ActivationFunctionType.Lrelu

_CACHED = None


def _build(S=S, NBLK=NBLK):
    TOK = BL * S
    nc = bacc.Bacc("TRN2", target_bir_lowering=False, debug=False,
                   num_devices=NCORES)

    xt_d = nc.dram_tensor("xt", [I, TOK], F32, kind="ExternalInput")
    wit_d = nc.dram_tensor("wit", [I, H], F32, kind="ExternalInput")
    wh1t_d = nc.dram_tensor("wh1t", [H, H], F32, kind="ExternalInput")
    wh2t_d = nc.dram_tensor("wh2t", [H, H], F32, kind="ExternalInput")
    wot_d = nc.dram_tensor("wot", [H, O], F32, kind="ExternalInput")
    bi_d = nc.dram_tensor("bi", [128, H // 128], F32, kind="ExternalInput")
    bh_d = nc.dram_tensor("bh", [1, H], F32, kind="ExternalInput")
    bo_d = nc.dram_tensor("bo", [1, O], F32, kind="ExternalInput")
    eye_d = nc.dram_tensor("eye128", [128, 128], F32, kind="ExternalInput")
    ident_d = nc.dram_tensor("ident", [16, 16], F32, kind="ExternalInput")
    ones_d = nc.dram_tensor("ones", [1, 128], F32, kind="ExternalInput")
    y_d = nc.dram_tensor("y", [BL, O], F32, kind="ExternalOutput")
    u_dram = nc.dram_tensor("udram", [TOK, H], F32R)

    with tile.TileContext(nc) as tc, ExitStack() as ctx:
        wpool = ctx.enter_context(tc.tile_pool(name="weights", bufs=1))
        xtpool = ctx.enter_context(tc.tile_pool(name="xt", bufs=3))
        apool = ctx.enter_context(tc.tile_pool(name="atiles", bufs=2))
        upool = ctx.enter_context(tc.tile_pool(name="usb", bufs=4))
        ringpool = ctx.enter_context(tc.tile_pool(name="uring", bufs=4))
        hpool = ctx.enter_context(tc.tile_pool(name="hbuf", bufs=2))
        opool = ctx.enter_context(tc.tile_pool(name="osb", bufs=1))
        ps1ctx = ExitStack()
        psA = ps1ctx.enter_context(tc.tile_pool(name="psA", bufs=2, space="PSUM"))
        psU = ps1ctx.enter_context(tc.tile_pool(name="psU", bufs=4, space="PSUM"))

        # ---- resident weights (gpsimd DMA casts f32 -> f32r/bf16) ----
        def wload(src, shape, tag, dt=F32R):
            t = wpool.tile(shape, dt, tag=tag, name=tag)
            nc.gpsimd.dma_start(t[:], src)
            return t

        wit = [wload(wit_d.ap()[128 * k:128 * (k + 1), :], [128, H], f"wit{k}")
               for k in range(2)]
        wh1t = [wload(wh1t_d.ap()[128 * k:128 * (k + 1), :], [128, H], f"wh1t{k}")
                for k in range(8)]
        wh2t = [wload(wh2t_d.ap()[128 * k:128 * (k + 1), :], [128, H],
                      f"wh2t{k}", dt=BF16)
                for k in range(8)]
        wot = [wload(wot_d.ap()[128 * k:128 * (k + 1), :], [128, O],
                     f"wot{k}", dt=BF16)
               for k in range(8)]
        eye = wload(eye_d.ap(), [128, 128], "eye")
        identb = wload(ident_d.ap(), [16, 16], "identb", dt=BF16)
        bh2 = wload(bh_d.ap(), [1, H], "bh2")
        bo2 = wload(bo_d.ap(), [1, O], "bo2")
        ones = wload(ones_d.ap(), [1, 128], "ones")
        bi = wpool.tile([128, H // 128], F32, tag="bi", name="bi")
        nc.sync.dma_start(bi[:], bi_d.ap())

        # ---- phase 1: A_T = lrelu(WiT.T @ Xt + bi); U = A @ Wh1.T + bh ----
        for blk in range(NBLK):
            c0 = 512 * blk
            xt = [xtpool.tile([128, 512], F32R, tag=f"xt{k}", name=f"xt{k}_{blk}") for k in range(2)]
            for k in range(2):
                nc.gpsimd.dma_start(
                    xt[k][:], xt_d.ap()[128 * k:128 * (k + 1), c0:c0 + 512])
            a = []
            for m in range(8):
                pa = psA.tile([128, 512], F32, tag="psA", name=f"psA_{blk}_{m}")
                nc.tensor.matmul(pa[:], wit[0][:, 128 * m:128 * (m + 1)],
                                 xt[0][:], start=True, stop=False)
                nc.tensor.matmul(pa[:], wit[1][:, 128 * m:128 * (m + 1)],
                                 xt[1][:], start=False, stop=True)
                am = apool.tile([128, 512], F32R, tag=f"a{m}", name=f"a{m}_{blk}")
                nc.scalar.activation(am[:], pa[:], LRELU,
                                     bias=bi[:, m:m + 1], scale=1.0, alpha=ALPHA)
                a.append(am)
            for q in range(4):
                pu = [psU.tile([128, 512], F32, tag="psU", name=f"psU_{blk}_{q}_{n}")
                      for n in range(2)]
                for n in range(2):
                    nc.tensor.matmul(pu[n][:], ones[0:1, 0:128],
                                     bh2[0:1, 512 * n:512 * (n + 1)],
                                     start=True, stop=False)
                for k in range(8):
                    for n in range(2):
                        nc.tensor.matmul(
                            pu[n][:], a[k][:, 128 * q:128 * (q + 1)],
                            wh1t[k][:, 512 * n:512 * (n + 1)],
                            start=False, stop=(k == 7))
                for n in range(2):
                    usb = upool.tile([128, 512], F32R, tag="usb", name=f"usb_{blk}_{q}_{n}")
                    nc.vector.tensor_copy(usb[:], pu[n][:])
                    nc.sync.dma_start(
                        u_dram.ap()[c0 + 128 * q:c0 + 128 * (q + 1),
                                    512 * n:512 * (n + 1)], usb[:])

        # ---- phase 2: recurrence ----
        ps1ctx.close()
        psR = ctx.enter_context(tc.tile_pool(name="psR", bufs=4, space="PSUM"))
        psTr = ctx.enter_context(tc.tile_pool(name="psTr", bufs=4, space="PSUM"))
        # h state transposed. Chunks 0-3 (from ps0, which stops early) ride
        # the long-latency XBAR DMA transpose as one [128, 4, 16] quad-tile;
        # chunks 4-7 (from ps1, stopping at step end) use short-latency PE
        # transposes + DVE copies into [128, 16] tiles so the next step's
        # matmuls aren't left waiting on a DMA semaphore.
        hTpE = hpool.tile([128, 2, 16], BF16, tag="hTpE", name="hTpE_init")
        nc.gpsimd.memset(hTpE[:].bitcast(F32), 0.0)
        hTL = {}
        for j in (0, 1, 4, 5, 6, 7):
            tl = hpool.tile([128, 16], BF16, tag=f"hTL{j}", name=f"hTL{j}_init")
            nc.gpsimd.memset(tl[:].bitcast(F32), 0.0)
            hTL[j] = tl



        ring = None
        ps_next = None

        def emit_id_mms(t):
            g = t % RING_STEPS
            sel = eye[:, 16 * g:16 * (g + 1)]
            ps0 = psR.tile([16, 512], F32, tag="psR", name=f"psR0_{t}")
            ps1 = psR.tile([16, 512], F32, tag="psR", name=f"psR1_{t}")
            nc.tensor.matmul(ps0[:], sel, ring[:, 0:512],
                             start=True, stop=False)
            nc.tensor.matmul(ps1[:], sel, ring[:, 512:1024],
                             start=True, stop=False)
            return ps0, ps1

        def load_ring(t):
            ring_new = ringpool.tile([RING_STEPS * BL, H], F32R, tag="ring", name=f"ring_{t}")
            r0 = t * BL
            nc.sync.dma_start(ring_new[:], u_dram.ap()[r0:r0 + RING_STEPS * BL, :])
            return ring_new

        for t in range(S):
            if t == 0:
                ring = load_ring(0)
                ps0, ps1 = emit_id_mms(0)
            else:
                ps0, ps1 = ps_next
            # ps0 chain first (stops early -> feeds the XBAR path), then ps1
            for k in range(8):
                nc.tensor.matmul(ps0[:],
                                 hTpE[:, k - 2, :] if k in (2, 3) else hTL[k][:],
                                 wh2t[k][:, 0:512],
                                 start=False, stop=(k == 7))
            for k in range(8):
                nc.tensor.matmul(ps1[:],
                                 hTpE[:, k - 2, :] if k in (2, 3) else hTL[k][:],
                                 wh2t[k][:, 512:1024],
                                 start=False, stop=(k == 7))
            if t + 1 < S:
                nxt = t + 1
                if nxt % RING_STEPS == 0:
                    ring = load_ring(nxt)
                ps_next = emit_id_mms(nxt)
            # E half: ACT [16,512]; chunks 2-3 via XBAR (long latency, used
            # late next step), chunks 0-1 via PE transposes (used first)
            hnE = hpool.tile([16, 512], BF16, tag="hnE", name=f"hnE_{t}")
            nc.scalar.activation(hnE[:], ps0[:], LRELU,
                                 bias=0.0, scale=1.0, alpha=ALPHA)
            hTpE = hpool.tile([128, 2, 16], BF16, tag="hTpE", name=f"hTpE_{t}")
            nc.scalar.dma_start_transpose(hTpE[:], hnE[:, 256:512])
            hTL_new = {}

            def pe_transpose(j, src_ap):
                pt = psTr.tile([128, 16], BF16, tag="psTr", name=f"psTr{j}_{t}")
                nc.tensor.transpose(pt[:], src_ap, identb[0:16, 0:16])
                tl = hpool.tile([128, 16], BF16, tag=f"hTL{j}",
                                name=f"hTL{j}_{t}")
                nc.vector.tensor_copy(tl[:], pt[:])
                hTL_new[j] = tl

            for j in (0, 1):
                pe_transpose(j, hnE[:, 128 * j:128 * (j + 1)])
            # L half: 2x ACT [16,256] -> 4 PE transposes + DVE copies
            for c in range(2):
                hnL = hpool.tile([16, 256], BF16, tag=f"hnL{c}",
                                 name=f"hnL{c}_{t}")
                nc.scalar.activation(hnL[:], ps1[:, 256 * c:256 * (c + 1)],
                                     LRELU, bias=0.0, scale=1.0, alpha=ALPHA)
                for u in range(2):
                    pe_transpose(4 + 2 * c + u,
                                 hnL[:, 128 * u:128 * (u + 1)])
            hTL = hTL_new

        # ---- phase 3: out = h_S @ Wo.T + bo ----
        po = psR.tile([16, 512], F32, tag="psR", name="psO")
        nc.tensor.matmul(po[:, 0:O], ones[0:1, 0:16], bo2[0:1, :],
                         start=True, stop=False)
        for k in range(8):
            nc.tensor.matmul(po[:, 0:O],
                             hTpE[:, k - 2, :] if k in (2, 3) else hTL[k][:],
                             wot[k][:],
                             start=False, stop=(k == 7))
        osb = opool.tile([16, O], F32, tag="osb", name="osb")
        nc.vector.tensor_copy(osb[:], po[:, 0:O])
        nc.sync.dma_start(y_d.ap(), osb[:])

    nc.compile()
    return nc


def _prep_inputs(x, Wi, bi, Wh, bh, Wo, bo):
    shared = {
        "wit": np.ascontiguousarray(Wi.T),
        "wh1t": np.ascontiguousarray(Wh[:, :H].T),
        "wh2t": np.ascontiguousarray(Wh[:, H:].T),
        "wot": np.ascontiguousarray(Wo.T),
        "bi": np.ascontiguousarray(bi.reshape(H // 128, 128).T),
        "bh": bh.reshape(1, H).copy(),
        "bo": bo.reshape(1, O).copy(),
        "eye128": np.eye(128, dtype=np.float32),
        "ident": np.eye(16, dtype=np.float32),
        "ones": np.ones((1, 128), np.float32),
    }
    in_maps = []
    for c in range(NCORES):
        xc = x[BL * c:BL * (c + 1)]            # [16, S, I]
        xt = np.ascontiguousarray(
            xc.transpose(2, 1, 0).reshape(I, TOK))  # [I, S*16] col = t*16+b
        m = dict(shared)
        m["xt"] = xt
        in_maps.append(m)
    return in_maps


def kernel(x, Wi, bi, Wh, bh, Wo, bo, _trace=False):
    global _CACHED
    x = np.asarray(x, dtype=np.float32)
    if _CACHED is None:
        _CACHED = _build()
    nc = _CACHED
    in_maps = _prep_inputs(np.asarray(x, np.float32), np.asarray(Wi, np.float32),
                           np.asarray(bi, np.float32), np.asarray(Wh, np.float32),
                           np.asarray(bh, np.float32), np.asarray(Wo, np.float32),
                           np.asarray(bo, np.float32))
    res = run_bass_kernel_spmd(nc, in_maps, list(range(NCORES)), trace=_trace)
    out = np.concatenate([res.results[c]["y"] for c in range(NCORES)], axis=0)
    if _trace:
        return out, res
    return out


# revision 41
# speedup vs baseline: 1.3293x; 1.0047x over previous
"""Trainium2 Bass kernel for a LeakyReLU RNN.

Model (B=128, S=512, I=256, H=1024, O=256):
    xproj = lrelu(x @ Wi.T + bi)                          # [B,S,H]
    h_t   = lrelu(concat(xproj_t, h_{t-1}) @ Wh.T + bh)   # recurrence over S
    out   = h_S @ Wo.T + bo                               # [B,O]

Strategy: data-parallel over batch (16 rows/core on 8 cores). Split
Wh = [Wh1 | Wh2]; U = xproj @ Wh1.T + bh is precomputed as big GEMMs,
the sequential part is h_t = lrelu(U_t + h_{t-1} @ Wh2.T) with the
hidden state as the (16-wide) stationary operand and Wh2.T streamed as
the moving operand.

The PE is instruction-issue-bound (~270ns per matmul regardless of
moving width up to 512), so the recurrence keeps only 18 PE
instructions per step: 2 identity matmuls injecting U_t into PSUM and
16 accumulating matmuls. The h-state transpose needed for the next
step's stationary operand is done off the PE with XBAR DMA transposes
(bf16), eliminating the 8 PE transposes + 8 vector copies per step
that dominated the previous version.
"""

from contextlib import ExitStack

import numpy as np

import concourse.bacc as bacc
import concourse.tile as tile
from concourse import mybir
from concourse.bass_utils import run_bass_kernel_spmd

B, S, I, H, O = 128, 512, 256, 1024, 256
NCORES = 8
BL = B // NCORES          # batch rows per core = 16
TOK = BL * S              # tokens per core = 8192
NBLK = TOK // 512         # 512-token blocks in phase 1 = 16
RING_STEPS = 8            # recurrence steps per U ring DMA
ALPHA = 0.01

F32 = mybir.dt.float32
F32R = mybir.dt.float32r
BF16 = mybir.dt.bfloat16
LRELU = mybir.ActivationFunctionType.Lrelu

_CACHED = None


def _build(S=S, NBLK=NBLK):
    TOK = BL * S
    nc = bacc.Bacc("TRN2", target_bir_lowering=False, debug=False,
                   num_devices=NCORES)

    xt_d = nc.dram_tensor("xt", [I, TOK], F32, kind="ExternalInput")
    wit_d = nc.dram_tensor("wit", [I, H], F32, kind="ExternalInput")
    wh1t_d = nc.dram_tensor("wh1t", [H, H], F32, kind="ExternalInput")
    wh2t_d = nc.dram_tensor("wh2t", [H, H], F32, kind="ExternalInput")
    wot_d = nc.dram_tensor("wot", [H, O], F32, kind="ExternalInput")
    bi_d = nc.dram_tensor("bi", [128, H // 128], F32, kind="ExternalInput")
    bh_d = nc.dram_tensor("bh", [1, H], F32, kind="ExternalInput")
    bo_d = nc.dram_tensor("bo", [1, O], F32, kind="ExternalInput")
    eye_d = nc.dram_tensor("eye128", [128, 128], F32, kind="ExternalInput")
    ident_d = nc.dram_tensor("ident", [16, 16], F32, kind="ExternalInput")
    ones_d = nc.dram_tensor("ones", [1, 128], F32, kind="ExternalInput")
    y_d = nc.dram_tensor("y", [BL, O], F32, kind="ExternalOutput")
    u_dram = nc.dram_tensor("udram", [TOK, H], F32R)

    with tile.TileContext(nc) as tc, ExitStack() as ctx:
        wpool = ctx.enter_context(tc.tile_pool(name="weights", bufs=1))
        xtpool = ctx.enter_context(tc.tile_pool(name="xt", bufs=3))
        apool = ctx.enter_context(tc.tile_pool(name="atiles", bufs=2))
        upool = ctx.enter_context(tc.tile_pool(name="usb", bufs=4))
        ringpool = ctx.enter_context(tc.tile_pool(name="uring", bufs=4))
        hpool = ctx.enter_context(tc.tile_pool(name="hbuf", bufs=2))
        opool = ctx.enter_context(tc.tile_pool(name="osb", bufs=1))
        ps1ctx = ExitStack()
        psA = ps1ctx.enter_context(tc.tile_pool(name="psA", bufs=3, space="PSUM"))
        psU = ps1ctx.enter_context(tc.tile_pool(name="psU", bufs=4, space="PSUM"))

        # ---- resident weights (gpsimd DMA casts f32 -> f32r/bf16) ----
        def wload(src, shape, tag, dt=F32R):
            t = wpool.tile(shape, dt, tag=tag, name=tag)
            nc.gpsimd.dma_start(t[:], src)
            return t

        wit = [wload(wit_d.ap()[128 * k:128 * (k + 1), :], [128, H], f"wit{k}")
               for k in range(2)]
        wh1t = [wload(wh1t_d.ap()[128 * k:128 * (k + 1), :], [128, H], f"wh1t{k}")
                for k in range(8)]
        wh2t = [wload(wh2t_d.ap()[128 * k:128 * (k + 1), :], [128, H],
                      f"wh2t{k}", dt=BF16)
                for k in range(8)]
        wot = [wload(wot_d.ap()[128 * k:128 * (k + 1), :], [128, O],
                     f"wot{k}", dt=BF16)
               for k in range(8)]
        eye = wload(eye_d.ap(), [128, 128], "eye")
        identb = wload(ident_d.ap(), [16, 16], "identb", dt=BF16)
        bh2 = wload(bh_d.ap(), [1, H], "bh2")
        bo2 = wload(bo_d.ap(), [1, O], "bo2")
        ones = wload(ones_d.ap(), [1, 128], "ones")
        bi = wpool.tile([128, H // 128], F32, tag="bi", name="bi")
        nc.sync.dma_start(bi[:], bi_d.ap())

        # ---- phase 1: A_T = lrelu(WiT.T @ Xt + bi); U = A @ Wh1.T + bh ----
        for blk in range(NBLK):
            c0 = 512 * blk
            xt = [xtpool.tile([128, 512], F32R, tag=f"xt{k}", name=f"xt{k}_{blk}") for k in range(2)]
            for k in range(2):
                nc.gpsimd.dma_start(
                    xt[k][:], xt_d.ap()[128 * k:128 * (k + 1), c0:c0 + 512])
            a = []
            for m in range(8):
                pa = psA.tile([128, 512], F32, tag="psA", name=f"psA_{blk}_{m}")
                nc.tensor.matmul(pa[:], wit[0][:, 128 * m:128 * (m + 1)],
                                 xt[0][:], start=True, stop=False)
                nc.tensor.matmul(pa[:], wit[1][:, 128 * m:128 * (m + 1)],
                                 xt[1][:], start=False, stop=True)
                am = apool.tile([128, 512], F32R, tag=f"a{m}", name=f"a{m}_{blk}")
                nc.scalar.activation(am[:], pa[:], LRELU,
                                     bias=bi[:, m:m + 1], scale=1.0, alpha=ALPHA)
                a.append(am)
            for q in range(4):
                pu = [psU.tile([128, 512], F32, tag="psU", name=f"psU_{blk}_{q}_{n}")
                      for n in range(2)]
                for n in range(2):
                    nc.tensor.matmul(pu[n][:], ones[0:1, 0:128],
                                     bh2[0:1, 512 * n:512 * (n + 1)],
                                     start=True, stop=False)
                for k in range(8):
                    for n in range(2):
                        nc.tensor.matmul(
                            pu[n][:], a[k][:, 128 * q:128 * (q + 1)],
                            wh1t[k][:, 512 * n:512 * (n + 1)],
                            start=False, stop=(k == 7))
                for n in range(2):
                    usb = upool.tile([128, 512], F32R, tag="usb", name=f"usb_{blk}_{q}_{n}")
                    nc.vector.tensor_copy(usb[:], pu[n][:])
                    nc.sync.dma_start(
                        u_dram.ap()[c0 + 128 * q:c0 + 128 * (q + 1),
                                    512 * n:512 * (n + 1)], usb[:])

        # ---- phase 2: recurrence ----
        ps1ctx.close()
        psR = ctx.enter_context(tc.tile_pool(name="psR", bufs=4, space="PSUM"))
        psTr = ctx.enter_context(tc.tile_pool(name="psTr", bufs=4, space="PSUM"))
        # h state transposed. Chunks 0-3 (from ps0, which stops early) ride
        # the long-latency XBAR DMA transpose as one [128, 4, 16] quad-tile;
        # chunks 4-7 (from ps1, stopping at step end) use short-latency PE
        # transposes + DVE copies into [128, 16] tiles so the next step's
        # matmuls aren't left waiting on a DMA semaphore.
        hTpE = hpool.tile([128, 2, 16], BF16, tag="hTpE", name="hTpE_init")
        nc.gpsimd.memset(hTpE[:].bitcast(F32), 0.0)
        hTL = {}
        for j in (0, 1, 4, 5, 6, 7):
            tl = hpool.tile([128, 16], BF16, tag=f"hTL{j}", name=f"hTL{j}_init")
            nc.gpsimd.memset(tl[:].bitcast(F32), 0.0)
            hTL[j] = tl



        ring = None
        ps_next = None

        def emit_id_mms(t):
            g = t % RING_STEPS
            sel = eye[:, 16 * g:16 * (g + 1)]
            ps0 = psR.tile([16, 512], F32, tag="psR", name=f"psR0_{t}")
            ps1 = psR.tile([16, 512], F32, tag="psR", name=f"psR1_{t}")
            nc.tensor.matmul(ps0[:], sel, ring[:, 0:512],
                             start=True, stop=False)
            nc.tensor.matmul(ps1[:], sel, ring[:, 512:1024],
                             start=True, stop=False)
            return ps0, ps1

        def load_ring(t):
            ring_new = ringpool.tile([RING_STEPS * BL, H], F32R, tag="ring", name=f"ring_{t}")
            r0 = t * BL
            nc.sync.dma_start(ring_new[:], u_dram.ap()[r0:r0 + RING_STEPS * BL, :])
            return ring_new

        for t in range(S):
            if t == 0:
                ring = load_ring(0)
                ps0, ps1 = emit_id_mms(0)
            else:
                ps0, ps1 = ps_next
            # ps0 chain first (stops early -> feeds the XBAR path), then ps1
            for k in range(8):
                nc.tensor.matmul(ps0[:],
                                 hTpE[:, k - 2, :] if k in (2, 3) else hTL[k][:],
                                 wh2t[k][:, 0:512],
                                 start=False, stop=(k == 7))
            for k in range(8):
                nc.tensor.matmul(ps1[:],
                                 hTpE[:, k - 2, :] if k in (2, 3) else hTL[k][:],
                                 wh2t[k][:, 512:1024],
                                 start=False, stop=(k == 7))
            if t + 1 < S:
                nxt = t + 1
                if nxt % RING_STEPS == 0:
                    ring = load_ring(nxt)
                ps_next = emit_id_mms(nxt)
            # E half: ACT [16,512]; chunks 2-3 via XBAR (long latency, used
            # late next step), chunks 0-1 via PE transposes (used first)
            hnE = hpool.tile([16, 512], BF16, tag="hnE", name=f"hnE_{t}")
            nc.scalar.activation(hnE[:], ps0[:], LRELU,
                                 bias=0.0, scale=1.0, alpha=ALPHA)
            hTpE = hpool.tile([128, 2, 16], BF16, tag="hTpE", name=f"hTpE_{t}")
            nc.scalar.dma_start_transpose(hTpE[:], hnE[:, 256:512])
            hTL_new = {}

            def pe_transpose(j, src_ap):
                pt = psTr.tile([128, 16], BF16, tag="psTr", name=f"psTr{j}_{t}")
                nc.tensor.transpose(pt[:], src_ap, identb[0:16, 0:16])
                tl = hpool.tile([128, 16], BF16, tag=f"hTL{j}",
                                name=f"hTL{j}_{t}")
                nc.vector.tensor_copy(tl[:], pt[:])
                hTL_new[j] = tl

            for j in (0, 1):
                pe_transpose(j, hnE[:, 128 * j:128 * (j + 1)])
            # L half: 2x ACT [16,256] -> 4 PE transposes + DVE copies
            for c in range(2):
                hnL = hpool.tile([16, 256], BF16, tag=f"hnL{c}",
                                 name=f"hnL{c}_{t}")
                nc.scalar.activation(hnL[:], ps1[:, 256 * c:256 * (c + 1)],
                                     LRELU, bias=0.0, scale=1.0, alpha=ALPHA)
                for u in range(2):
                    pe_transpose(4 + 2 * c + u,
                                 hnL[:, 128 * u:128 * (u + 1)])
            hTL = hTL_new

        # ---- phase 3: out = h_S @ Wo.T + bo ----
        po = psR.tile([16, 512], F32, tag="psR", name="psO")
        nc.tensor.matmul(po[:, 0:O], ones[0:1, 0:16], bo2[0:1, :],
                         start=True, stop=False)
        for k in range(8):
            nc.tensor.matmul(po[:, 0:O],
                             hTpE[:, k - 2, :] if k in (2, 3) else hTL[k][:],
                             wot[k][:],
                             start=False, stop=(k == 7))
        osb = opool.tile([16, O], F32, tag="osb", name="osb")
        nc.vector.tensor_copy(osb[:], po[:, 0:O])
        nc.sync.dma_start(y_d.ap(), osb[:])

    nc.compile()
    return nc


def _prep_inputs(x, Wi, bi, Wh, bh, Wo, bo):
    shared = {
        "wit": np.ascontiguousarray(Wi.T),
        "wh1t": np.ascontiguousarray(Wh[:, :H].T),
        "wh2t": np.ascontiguousarray(Wh[:, H:].T),
        "wot": np.ascontiguousarray(Wo.T),
        "bi": np.ascontiguousarray(bi.reshape(H // 128, 128).T),
        "bh": bh.reshape(1, H).copy(),
        "bo": bo.reshape(1, O).copy(),
        "eye128": np.eye(128, dtype=np.float32),
        "ident": np.eye(16, dtype=np.float32),
        "ones": np.ones((1, 128), np.float32),
    }
    in_maps = []
    for c in range(NCORES):
        xc = x[BL * c:BL * (c + 1)]            # [16, S, I]
        xt = np.ascontiguousarray(
            xc.transpose(2, 1, 0).reshape(I, TOK))  # [I, S*16] col = t*16+b
        m = dict(shared)
        m["xt"] = xt
        in_maps.append(m)
    return in_maps


def kernel(x, Wi, bi, Wh, bh, Wo, bo, _trace=False):
    global _CACHED
    x = np.asarray(x, dtype=np.float32)
    if _CACHED is None:
        _CACHED = _build()
    nc = _CACHED
    in_maps = _prep_inputs(np.asarray(x, np.float32), np.asarray(Wi, np.float32),
                           np.asarray(bi, np.float32), np.asarray(Wh, np.float32),
                           np.asarray(bh, np.float32), np.asarray(Wo, np.float32),
                           np.asarray(bo, np.float32))
    res = run_bass_kernel_spmd(nc, in_maps, list(range(NCORES)), trace=_trace)
    out = np.concatenate([res.results[c]["y"] for c in range(NCORES)], axis=0)
    if _trace:
        return out, res
    return out
